# revision 1
# baseline (speedup 1.0000x reference)
"""Trainium2 Bass kernel for nn_CascadedMartingaleEncoder.

Two-phase SPMD design over 8 NeuronCores:
  phase A: 3x3 texture-martingale statistics, sharded (batch x channel-half).
           THETA=1 collapses the martingale map to an affine rescale folded
           into the conv weights; the energy/entropy box filters commute
           with the 1x1 conv and are folded into the bilinear-resize
           matrices on host; the contrast feature is 8/9 + O(eps/sqrt(S2))
           (<= 6e-5 for uniform inputs) and is folded into the per-channel
           constant. Homogeneity uses sum|x-mu| = 2*sum relu(x-mu) (the 3x3
           window sum of x-mu is zero), computed in bf16.
  phase B: fused 1x1 conv + layernorm (ones-matmul trick for the
           partition-axis reduction) + 8-head attention in float32r,
           sharded (batch x query-token-half) with host-side token rolling
           so the SPMD program is identical on every core.
"""

import numpy as np



import concourse.bass as bass
import concourse.tile as tile
import concourse.mybir as mybir
from concourse import bacc

F32 = mybir.dt.float32
AF = mybir.ActivationFunctionType
ALU = mybir.AluOpType
EPS = 1e-6
E5 = float(np.exp(-0.5))

NCORES = 8
H = W = 64
HW = H * W
S = 1024         # tokens in phase B
D = 512
NH = 8
DH = 64


def new_bass():
    return bacc.Bacc("TRN2", target_bir_lowering=False, debug=False,
                     num_devices=NCORES)


def compile_with_pinned_tables(nc):
    """Restrict ACT table-set choice to {natural_log_exp_and_others,
    sqrt_and_others} (positions preserved) so Ln/Exp never thrash sets."""
    import concourse.bacc as bacc_mod
    keep = {"natural_log_exp_and_others", "sqrt_and_others"}
    orig = bacc_mod.get_activation_tables

    def pinned(arch):
        return {name: (fns if name in keep else set())
                for name, fns in orig(arch).items()}

    bacc_mod.get_activation_tables = pinned
    try:
        nc.compile()
    finally:
        bacc_mod.get_activation_tables = orig
    return nc


# --------------------------------------------------------------------------
# Phase A kernel: per-core 128 channels, full 64x64 spatial.
# Outputs (per core, all [12, 4096] f32, pre-downsample):
#   contribA = sum_c Wc'*contrast_raw + Wh'*hom   (contrast_raw = S2*rc = 9*feat)
#   qe       = sum_c We'*x^2        (box9 + resize applied on host)
#   qent     = sum_c Wen'*x*ln(x+eps)
# --------------------------------------------------------------------------
def build_phase_a():
    nc = new_bass()
    x_in = nc.dram_tensor("x_sh", [128, H, W], F32, kind="ExternalInput")
    xb_in = nc.dram_tensor("x_sh_bf", [128, H, W], mybir.dt.bfloat16,
                           kind="ExternalInput")
    w_in = nc.dram_tensor("wstack", [128, 48], F32, kind="ExternalInput")
    o_a = nc.dram_tensor("contribA", [12, HW], F32, kind="ExternalOutput")
    o_e = nc.dram_tensor("qe", [12, HW], F32, kind="ExternalOutput")
    o_n = nc.dram_tensor("qent", [12, HW], F32, kind="ExternalOutput")
    BF = mybir.dt.bfloat16

    with tile.TileContext(nc) as tc:
        with (
            tc.tile_pool(name="const", bufs=1) as constp,
            tc.tile_pool(name="pad", bufs=1) as padp,
            tc.tile_pool(name="big", bufs=1) as bigp,
            tc.tile_pool(name="stage", bufs=3) as stagep,
            tc.tile_pool(name="ps", bufs=4, space="PSUM") as psp,
        ):
            wst = constp.tile([128, 48], F32, tag="w")
            nc.sync.dma_start(out=wst, in_=w_in[:, :])

            def constv(value, p=128):
                t = constp.tile([p, 1], F32, tag=f"c{value}")
                nc.vector.memset(t, value)
                return t

            c_eps = constv(EPS)

            def padded(tag, dtype=F32, wpad=W + 2):
                t = padp.tile([128, H + 2, wpad], dtype, tag=tag)
                nc.gpsimd.memset(t[:, 0, :], 0.0)
                nc.gpsimd.memset(t[:, H + 1, :], 0.0)
                nc.gpsimd.memset(t[:, 1:H + 1, 0], 0.0)
                nc.gpsimd.memset(t[:, 1:H + 1, W + 1:wpad], 0.0)
                return t

            x66 = padded("x66")
            nc.sync.dma_start(out=x66[:, 1:H + 1, 1:W + 1], in_=x_in[:, :, :])
            xin = x66[:, 1:H + 1, 1:W + 1]  # inner view

            x_bf = padded("x_bf", dtype=BF, wpad=W + 4)
            nc.sync.dma_start(out=x_bf[:, 1:H + 1, 1:W + 1], in_=xb_in[:, :, :])

            # x^2 on the full padded tile: border stays zero (0^2 = 0)
            x2_66 = padp.tile([128, H + 2, W + 2], F32, tag="x2_66")
            nc.scalar.activation(out=x2_66[:, :, :], in_=x66[:, :, :], func=AF.Square)

            # xlx = x * ln(x + eps)   (GPSIMD does the multiply)
            lx = bigp.tile([128, H, W], F32, tag="boxtmp")
            nc.scalar.activation(out=lx, in_=xin, func=AF.Ln, bias=c_eps)
            xlx = bigp.tile([128, H, W], F32, tag="xlx")
            nc.gpsimd.tensor_tensor(out=xlx, in0=xin, in1=lx, op=ALU.mult)

            # ---- contractions that commute with box9: qe (x^2), qent (xlx)
            def contract(lhsT, rhs_chunk_fn, out_dram, extra_lhsT=None,
                         extra_rhs_fn=None):
                for j in range(8):
                    ps = psp.tile([12, 512], F32, tag="ps", name=f"ps{j}")
                    nc.tensor.matmul(ps, lhsT, rhs_chunk_fn(j),
                                     start=True, stop=(extra_lhsT is None))
                    if extra_lhsT is not None:
                        nc.tensor.matmul(ps, extra_lhsT, extra_rhs_fn(j),
                                         start=False, stop=True)
                    st = stagep.tile([12, 512], F32, tag="st", name=f"st{j}")
                    nc.scalar.activation(out=st, in_=ps, func=AF.Copy)
                    nc.sync.dma_start(out=out_dram[:, 512 * j:512 * (j + 1)], in_=st)

            contract(wst[:, 12:24],
                     lambda j: x2_66[:, 1 + 8 * j:9 + 8 * j, 1:W + 1], o_e)
            contract(wst[:, 24:36],
                     lambda j: xlx[:, 8 * j:8 * (j + 1), :], o_n)

            # ---- box9(x) on DVE, box9(x^2) on GPSIMD (parallel engines)
            def box9(src66, dst_tag, eng):
                tmp = bigp.tile([128, H, W], F32, tag="boxtmp", name=f"tmp_{dst_tag}")
                eng.tensor_tensor(
                    out=tmp, in0=src66[:, 1:H + 1, 0:W],
                    in1=src66[:, 1:H + 1, 2:W + 2], op=ALU.add)
                bw = padded(f"bw_{dst_tag}")
                eng.tensor_tensor(
                    out=bw[:, 1:H + 1, 1:W + 1], in0=tmp,
                    in1=src66[:, 1:H + 1, 1:W + 1], op=ALU.add)
                tmp2 = bigp.tile([128, H, W], F32, tag="boxtmp", name=f"tmp2_{dst_tag}")
                eng.tensor_tensor(
                    out=tmp2, in0=bw[:, 0:H, 1:W + 1],
                    in1=bw[:, 2:H + 2, 1:W + 1], op=ALU.add)
                dst = bigp.tile([128, H, W], F32, tag=dst_tag, name=f"dst_{dst_tag}")
                eng.tensor_tensor(
                    out=dst, in0=tmp2, in1=bw[:, 1:H + 1, 1:W + 1], op=ALU.add)
                return dst

            # box9 of x in bf16 (valid since contrast/S2 no longer exist:
            # S1 only feeds the bf16 mean). bw inner sits at col 2 so every
            # pass is 4B-aligned for the DVE 2x bf16 mode.
            tmpb = bigp.tile([128, H, W], BF, tag="boxtmp", name="tmpb")
            nc.vector.tensor_tensor(
                out=tmpb, in0=x_bf[:, 1:H + 1, 0:W],
                in1=x_bf[:, 1:H + 1, 2:W + 2], op=ALU.add)
            bwb = padded("bw_S1", dtype=BF, wpad=W + 4)
            nc.vector.tensor_tensor(
                out=bwb[:, 1:H + 1, 2:W + 2], in0=tmpb,
                in1=x_bf[:, 1:H + 1, 1:W + 1], op=ALU.add)
            tmp2b = bigp.tile([128, H, W], BF, tag="boxtmp", name="tmp2b")
            nc.vector.tensor_tensor(
                out=tmp2b, in0=bwb[:, 0:H, 2:W + 2],
                in1=bwb[:, 2:H + 2, 2:W + 2], op=ALU.add)
            S1 = bigp.tile([128, H, W], BF, tag="S1", name="S1")
            nc.vector.tensor_tensor(
                out=S1, in0=tmp2b, in1=bwb[:, 1:H + 1, 2:W + 2], op=ALU.add)
            # Contrast feat (S2/9)/(sqrt(S2/8)+eps)^2 deviates from 8/9 by
            # O(eps/sqrt(S2)) <= 6e-5 for uniform inputs; it is folded into
            # the host-side per-channel constant instead of being computed.

            # ---- homogeneity in bf16 (relu form): u_in = sum_k relu(x_k - mean)
            # x_bf uses W-stride 68 so taps with even col offset hit the DVE
            # 2x bf16 mode; the dw=1 taps (odd offset) run on GPSIMD instead.
            mean_bf = bigp.tile([128, H, W], BF, tag="mean")
            nc.scalar.activation(out=mean_bf, in_=S1, func=AF.Copy, scale=1.0 / 9.0)
            G = bigp.tile([128, H, W], BF, tag="G")
            gps_parts = []
            first = True
            for dh in range(3):
                for dw in range(3):
                    dk = bigp.tile([128, H, W], BF, tag="dk", bufs=3,
                                   name=f"dk{dh}{dw}")
                    nc.vector.tensor_tensor(
                        out=dk, in0=x_bf[:, dh:dh + H, dw:dw + W], in1=mean_bf,
                        op=ALU.subtract)
                    if first:
                        nc.vector.tensor_relu(out=G, in_=dk)
                        first = False
                    else:
                        nc.vector.tensor_relu(out=dk, in_=dk)
                        nc.vector.tensor_tensor(out=G, in0=G, in1=dk, op=ALU.add)
            u = bigp.tile([128, H, W], F32, tag="scrA", name="u")
            nc.scalar.activation(out=u, in_=G, func=AF.Ln, scale=2.0 / 9.0, bias=1.0)
            hom = u  # in-place
            nc.scalar.activation(out=hom, in_=u, func=AF.Exp, scale=-1.0)

            # ---- contribA = Wh'.T @ hom
            contract(wst[:, 36:48],
                     lambda j: hom[:, 8 * j:8 * (j + 1), :], o_a)

    compile_with_pinned_tables(nc)
    return nc


# --------------------------------------------------------------------------
# Host-side helpers
# --------------------------------------------------------------------------
def resize_mat(out_size, in_size):
    scale = out_size / in_size
    sample_f = (np.arange(out_size) + 0.5) / scale - 0.5
    x = np.abs(sample_f[:, None] - np.arange(in_size)[None, :]) * scale
    w = np.maximum(0.0, 1.0 - x)
    w = w / w.sum(axis=1, keepdims=True)
    return w  # float64 [out, in]


def prep_a(x, reduce_w):
    """Returns in_maps (list of 8 dicts) for phase A."""
    E = E5
    in_maps = []
    for core in range(NCORES):
        b, half = core // 2, core % 2
        c0 = half * 128
        cols = 4 * (c0 + np.arange(128))
        wstack = np.stack([
            reduce_w[:, cols + 0].T * (E / 9.0),      # contrast
            reduce_w[:, cols + 1].T * (E / 9.0),      # energy (pre-box)
            reduce_w[:, cols + 2].T * (-E / 9.0),     # entropy (pre-box)
            reduce_w[:, cols + 3].T * E,              # homogeneity
        ], axis=1).reshape(128, 48).astype(np.float32)
        # stack axis order: [128, 4stats, 12] -> [128, 48] with stat-major cols
        import ml_dtypes
        in_maps.append({
            "x_sh": np.ascontiguousarray(x[b, c0:c0 + 128]).astype(np.float32),
            "x_sh_bf": np.ascontiguousarray(x[b, c0:c0 + 128]).astype(ml_dtypes.bfloat16),
            "wstack": np.ascontiguousarray(wstack),
        })
    return in_maps


def combine_a(results, reduce_w, reduce_b):
    """results: list of 8 dicts with contribA/qe/qent [12, 4096].
    Returns m_up [4, 12, 32, 32] float32."""
    R = resize_mat(32, 64)
    Bm = (np.abs(np.arange(64)[:, None] - np.arange(64)[None, :]) <= 1).astype(np.float64)
    RB = R @ Bm
    rw = reduce_w.astype(np.float64)
    const_o = (reduce_b.astype(np.float64) + EPS * E5 * rw.sum(axis=1)
               + (8.0 / 9.0) * E5 * rw[:, 0::4].sum(axis=1))
    m_up = np.zeros((4, 12, 32, 32), np.float64)
    for b in range(4):
        A = (results[2 * b]["contribA"].astype(np.float64)
             + results[2 * b + 1]["contribA"].astype(np.float64)).reshape(12, 64, 64)
        qe = (results[2 * b]["qe"].astype(np.float64)
              + results[2 * b + 1]["qe"].astype(np.float64)).reshape(12, 64, 64)
        qn = (results[2 * b]["qent"].astype(np.float64)
              + results[2 * b + 1]["qent"].astype(np.float64)).reshape(12, 64, 64)
        m_up[b] = (np.einsum("ij,ojk,lk->oil", R, A, R)
                   + np.einsum("ij,ojk,lk->oil", RB, qe, RB)
                   + np.einsum("ij,ojk,lk->oil", RB, qn, RB)
                   + const_o[:, None, None])
    return m_up.astype(np.float32)


# --------------------------------------------------------------------------
# Phase B kernel: fuse conv + LN + 8-head attention for one batch element,
# query-token half = tokens [0:512] (host pre-rolls tokens per core).
# --------------------------------------------------------------------------
def build_phase_b():
    FR = mybir.dt.float32r
    nc = new_bass()
    xn_in = nc.dram_tensor("xn", [512, S], FR, kind="ExternalInput")
    mu_in = nc.dram_tensor("mu12", [12, S], FR, kind="ExternalInput")
    wxt_in = nc.dram_tensor("wxt", [512, 512], FR, kind="ExternalInput")
    wmt_in = nc.dram_tensor("wmt", [12, 512], FR, kind="ExternalInput")
    wpt_in = nc.dram_tensor("wpt", [12, 512], FR, kind="ExternalInput")
    wqt_in = nc.dram_tensor("wqt", [512, 512], FR, kind="ExternalInput")
    wkt_in = nc.dram_tensor("wkt", [512, 512], FR, kind="ExternalInput")
    wvt_in = nc.dram_tensor("wvt", [512, 512], FR, kind="ExternalInput")
    wot_in = nc.dram_tensor("wot", [512, 512], FR, kind="ExternalInput")
    bp_in = nc.dram_tensor("biasp", [128, 20], F32, kind="ExternalInput")
    bv_in = nc.dram_tensor("bvrow", [1, 512], FR, kind="ExternalInput")
    ones_in = nc.dram_tensor("ones128", [128, 128], FR, kind="ExternalInput")
    fout = nc.dram_tensor("fout", [512, 512], F32, kind="ExternalOutput")

    SQ = 512  # query tokens per core
    with tile.TileContext(nc) as tc:
        with (
            tc.tile_pool(name="wp", bufs=1) as wp,
            tc.tile_pool(name="dat", bufs=1) as dat,
            tc.tile_pool(name="scr", bufs=2) as scr,
            tc.tile_pool(name="ps", bufs=8, space="PSUM") as psp,
        ):
            def load(pool, dram, shape, tag, bufs=1, dtype=None):
                t = pool.tile(shape, dtype or FR, tag=tag, bufs=bufs)
                nc.sync.dma_start(out=t, in_=dram[tuple(slice(None) for _ in shape)])
                return t

            biasp = load(wp, bp_in, [128, 20], "biasp", dtype=F32)
            bvrow = load(wp, bv_in, [1, 512], "bvrow")
            mu_t = load(wp, mu_in, [12, S], "mu")
            wmt_t = load(wp, wmt_in, [12, 512], "wmt")
            wpt_t = load(wp, wpt_in, [12, 512], "wpt")
            ones128 = load(wp, ones_in, [128, 128], "ones128")
            ones1 = ones128[0:1, :]
            c_ln5 = wp.tile([128, 1], F32, tag="c_ln5")
            nc.vector.memset(c_ln5, 1e-5)

            def load_blocks(dram, tag):
                ts = []
                for kb in range(4):
                    t = wp.tile([128, 512], FR, tag=f"{tag}{kb}")
                    nc.sync.dma_start(out=t, in_=dram[128 * kb:128 * (kb + 1), :])
                    ts.append(t)
                return ts

            wxt_t = load_blocks(wxt_in, "wxt")
            wqt_t = load_blocks(wqt_in, "wqt")
            wkt_t = load_blocks(wkt_in, "wkt")
            wvt_t = load_blocks(wvt_in, "wvt")
            wot_t = load_blocks(wot_in, "wot")

            xn_t = []
            for kb in range(4):
                t = dat.tile([128, S], FR, tag="big1024", bufs=8)
                nc.sync.dma_start(out=t, in_=xn_in[128 * kb:128 * (kb + 1), :])
                xn_t.append(t)

            # ---- fuse conv: v_sb = xf (V source), qkp = xf + proj@mu + proj_b
            v_sb = [[None, None] for _ in range(4)]
            qkp = [[None, None] for _ in range(4)]
            for mb in range(4):
                psA = [psp.tile([128, 512], F32, tag="ps", bufs=8, name=f"psA{mb}_{i}") for i in range(2)]
                for kb in range(4):
                    for nb in range(2):
                        nc.tensor.matmul(
                            psA[nb], wxt_t[kb][:, 128 * mb:128 * (mb + 1)],
                            xn_t[kb][:, 512 * nb:512 * (nb + 1)],
                            start=(kb == 0), stop=False)
                for nb in range(2):
                    nc.tensor.matmul(
                        psA[nb], wmt_t[:, 128 * mb:128 * (mb + 1)],
                        mu_t[:, 512 * nb:512 * (nb + 1)],
                        start=False, stop=True)
                for nb in range(2):
                    v = dat.tile([128, 512], FR, tag="v_sb", bufs=8)
                    nc.scalar.activation(out=v, in_=psA[nb], func=AF.Identity,
                                         bias=biasp[:, mb:mb + 1])
                    v_sb[mb][nb] = v
                    psB = psp.tile([128, 512], F32, tag="ps", bufs=8)
                    nc.tensor.matmul(
                        psB, wpt_t[:, 128 * mb:128 * (mb + 1)],
                        mu_t[:, 512 * nb:512 * (nb + 1)], start=True, stop=True)
                    q = dat.tile([128, 512], FR, tag="qkp", bufs=8)
                    nc.vector.scalar_tensor_tensor(
                        out=q, in0=psB, scalar=biasp[:, 4 + mb:5 + mb], in1=v,
                        op0=ALU.add, op1=ALU.add)
                    qkp[mb][nb] = q

            # ---- LN over channels (partition dim) via ones-matmul
            psmu = [psp.tile([128, 512], F32, tag="ps", bufs=8, name=f"psmu{i}") for i in range(2)]
            pss2 = [psp.tile([128, 512], F32, tag="ps", bufs=8, name=f"pss2{i}") for i in range(2)]
            for nb in range(2):
                for mb in range(4):
                    nc.tensor.matmul(psmu[nb], ones128, qkp[mb][nb],
                                     start=(mb == 0), stop=(mb == 3))
                for mb in range(4):
                    sq = scr.tile([128, 512], FR, tag="sq", bufs=2)
                    nc.scalar.activation(out=sq, in_=qkp[mb][nb], func=AF.Square)
                    nc.tensor.matmul(pss2[nb], ones128, sq,
                                     start=(mb == 0), stop=(mb == 3))
            mu_sb, rstd = [], []
            for nb in range(2):
                m = dat.tile([128, 512], F32, tag="mu_sb", bufs=2)
                nc.vector.tensor_scalar_mul(m, psmu[nb], 1.0 / 512.0)
                mu_sb.append(m)
                musq = scr.tile([128, 512], F32, tag="musq", bufs=2)
                nc.vector.tensor_tensor(out=musq, in0=m, in1=m, op=ALU.mult)
                vt = scr.tile([128, 512], F32, tag="vt", bufs=2)
                nc.vector.scalar_tensor_tensor(
                    out=vt, in0=musq, scalar=-512.0, in1=pss2[nb],
                    op0=ALU.mult, op1=ALU.add)
                lnv = scr.tile([128, 512], F32, tag="lnv", bufs=2)
                nc.scalar.activation(out=lnv, in_=vt, func=AF.Ln,
                                     scale=1.0 / 512.0, bias=c_ln5)
                r = dat.tile([128, 512], F32, tag="rstd", bufs=2)
                nc.scalar.activation(out=r, in_=lnv, func=AF.Exp, scale=-0.5)
                rstd.append(r)
            for mb in range(4):
                for nb in range(2):
                    nc.vector.scalar_tensor_tensor(
                        out=qkp[mb][nb], in0=mu_sb[nb], scalar=-1.0,
                        in1=qkp[mb][nb], op0=ALU.mult, op1=ALU.add)
                    nc.vector.tensor_tensor(out=qkp[mb][nb], in0=qkp[mb][nb],
                                            in1=rstd[nb], op=ALU.mult)
            qkl = qkp  # renamed: now layer-normed

            # ---- Q (tokens 0:512 only), K (all tokens)
            q_sb = []
            for hp in range(4):
                psq = psp.tile([128, 512], F32, tag="ps", bufs=8)
                for kb in range(4):
                    nc.tensor.matmul(psq, wqt_t[kb][:, 128 * hp:128 * (hp + 1)],
                                     qkl[kb][0], start=(kb == 0), stop=(kb == 3))
                qt = dat.tile([128, 512], FR, tag="q_sb", bufs=4)
                nc.scalar.activation(out=qt, in_=psq, func=AF.Identity,
                                     bias=biasp[:, 8 + hp:9 + hp])
                q_sb.append(qt)
            k_sb = []
            for hp in range(4):
                kt = dat.tile([128, S], FR, tag="big1024", bufs=8)
                for nb in range(2):
                    psk = psp.tile([128, 512], F32, tag="ps", bufs=8)
                    for kb in range(4):
                        nc.tensor.matmul(psk, wkt_t[kb][:, 128 * hp:128 * (hp + 1)],
                                         qkl[kb][nb], start=(kb == 0), stop=(kb == 3))
                    nc.scalar.activation(out=kt[:, 512 * nb:512 * (nb + 1)], in_=psk,
                                         func=AF.Identity, bias=biasp[:, 12 + hp:13 + hp])
                k_sb.append(kt)

            # ---- V in [token, head-dim] layout (bias via ones1 outer product)
            v2_sb = []
            for tb in range(8):
                psv = psp.tile([128, 512], F32, tag="ps", bufs=8)
                nc.tensor.matmul(psv, ones1, bvrow, start=True, stop=False)
                for kb in range(4):
                    nc.tensor.matmul(
                        psv,
                        v_sb[kb][tb // 4][:, 128 * (tb % 4):128 * (tb % 4 + 1)],
                        wvt_t[kb], start=False, stop=(kb == 3))
                vt2 = dat.tile([128, 512], FR, tag="v2_sb", bufs=8)
                nc.vector.tensor_copy(vt2, psv)
                v2_sb.append(vt2)

            # ---- attention per head
            o_sb = [dat.tile([128, SQ], FR, tag="o_sb", bufs=4, name=f"o_sb{i}") for i in range(4)]
            for h in range(NH):
                hp, off = h // 2, 64 * (h % 2)
                e_sb = []
                for tb in range(8):
                    pss = psp.tile([128, SQ], F32, tag="ps", bufs=8)
                    nc.tensor.matmul(
                        pss, k_sb[hp][off:off + 64, 128 * tb:128 * (tb + 1)],
                        q_sb[hp][off:off + 64, :], start=True, stop=True)
                    e = dat.tile([128, SQ], FR, tag="e_sb", bufs=12)
                    nc.scalar.activation(out=e, in_=pss, func=AF.Exp)
                    e_sb.append(e)
                pssum = psp.tile([128, SQ], F32, tag="ps", bufs=8)
                for tb in range(8):
                    nc.tensor.matmul(pssum, ones128, e_sb[tb],
                                     start=(tb == 0), stop=(tb == 7))
                lns = scr.tile([128, SQ], F32, tag="lns", bufs=2)
                nc.scalar.activation(out=lns, in_=pssum, func=AF.Ln)
                rinv = scr.tile([128, SQ], F32, tag="rinv", bufs=2)
                nc.scalar.activation(out=rinv, in_=lns, func=AF.Exp, scale=-1.0)
                pso = psp.tile([64, SQ], F32, tag="ps", bufs=8)
                for tb in range(8):
                    nc.tensor.matmul(pso, v2_sb[tb][:, 64 * h:64 * (h + 1)],
                                     e_sb[tb], start=(tb == 0), stop=(tb == 7))
                nc.vector.tensor_tensor(out=o_sb[hp][off:off + 64, :], in0=pso,
                                        in1=rinv[0:64, :], op=ALU.mult)

            # ---- output projection
            for mb in range(4):
                psf = psp.tile([128, SQ], F32, tag="ps", bufs=8)
                for hp in range(4):
                    nc.tensor.matmul(psf, wot_t[hp][:, 128 * mb:128 * (mb + 1)],
                                     o_sb[hp], start=(hp == 0), stop=(hp == 3))
                fo = scr.tile([128, SQ], F32, tag="fo", bufs=2)
                nc.scalar.activation(out=fo, in_=psf, func=AF.Identity,
                                     bias=biasp[:, 16 + mb:17 + mb])
                nc.sync.dma_start(out=fout[128 * mb:128 * (mb + 1), :], in_=fo)

    compile_with_pinned_tables(nc)
    return nc


def prep_b(x_next, m_up, fuse_w, fuse_b, proj_w, proj_b, ln_g, ln_b,
           in_proj_w, in_proj_b, out_proj_w, out_proj_b):
    wq, wk, wv = in_proj_w[:512], in_proj_w[512:1024], in_proj_w[1024:]
    bq, bk, bv = in_proj_b[:512], in_proj_b[512:1024], in_proj_b[1024:]
    f32 = np.float32
    wq2 = (wq * ln_g[None, :]) / 8.0
    bq2 = (bq + wq @ ln_b) / 8.0
    wk2 = wk * ln_g[None, :]
    bk2 = bk + wk @ ln_b

    def pack(v):
        return np.ascontiguousarray(v.reshape(4, 128).T).astype(f32)

    biasp = np.concatenate([pack(fuse_b), pack(proj_b), pack(bq2), pack(bk2),
                            pack(out_proj_b)], axis=1)
    shared = {
        "wxt": np.ascontiguousarray(fuse_w[:, :512].T).astype(f32),
        "wmt": np.ascontiguousarray(fuse_w[:, 512:].T).astype(f32),
        "wpt": np.ascontiguousarray(proj_w.T).astype(f32),
        "wqt": np.ascontiguousarray(wq2.T).astype(f32),
        "wkt": np.ascontiguousarray(wk2.T).astype(f32),
        "wvt": np.ascontiguousarray(wv.T).astype(f32),
        "wot": np.ascontiguousarray(out_proj_w.T).astype(f32),
        "biasp": biasp,
        "bvrow": np.ascontiguousarray(bv[None, :]).astype(f32),
        "ones128": np.ones((128, 128), f32),
    }
    in_maps = []
    for core in range(NCORES):
        b, half = core // 2, core % 2
        xn = x_next[b].reshape(512, S)
        mu = m_up[b].reshape(12, S)
        if half == 1:
            xn = np.roll(xn, -512, axis=1)
            mu = np.roll(mu, -512, axis=1)
        m = dict(shared)
        m["xn"] = np.ascontiguousarray(xn).astype(f32)
        m["mu12"] = np.ascontiguousarray(mu).astype(f32)
        in_maps.append(m)
    return in_maps


def combine_b(results):
    out = np.zeros((4, 512, S), np.float32)
    for core in range(NCORES):
        b, half = core // 2, core % 2
        out[b][:, 512 * half:512 * (half + 1)] = results[core]["fout"]
    return out.reshape(4, 512, 32, 32)


# --------------------------------------------------------------------------
# Entry point
# --------------------------------------------------------------------------
_CACHE = {}


def _get_programs():
    if "a" not in _CACHE:
        _CACHE["a"] = build_phase_a()
        _CACHE["b"] = build_phase_b()
    return _CACHE["a"], _CACHE["b"]


def kernel(x, x_next, reduce_w, reduce_b, fuse_w, fuse_b, proj_w, proj_b,
           ln_g, ln_b, in_proj_w, in_proj_b, out_proj_w, out_proj_b):
    from concourse.bass_utils import run_bass_kernel_spmd

    x = np.asarray(x, np.float32)
    x_next = np.asarray(x_next, np.float32)
    nc_a, nc_b = _get_programs()

    in_a = prep_a(x, np.asarray(reduce_w, np.float32))
    res_a = run_bass_kernel_spmd(nc_a, in_a, core_ids=list(range(NCORES)))
    m_up = combine_a(res_a.results, np.asarray(reduce_w, np.float64),
                     np.asarray(reduce_b, np.float64))

    in_b = prep_b(x_next, m_up, *(np.asarray(t, np.float32) for t in (
        fuse_w, fuse_b, proj_w, proj_b, ln_g, ln_b,
        in_proj_w, in_proj_b, out_proj_w, out_proj_b)))
    res_b = run_bass_kernel_spmd(nc_b, in_b, core_ids=list(range(NCORES)))
    return combine_b(res_b.results)



# revision 14
# speedup vs baseline: 1.2899x; 1.2899x over previous
"""Trainium2 Bass kernel for nn_CascadedMartingaleEncoder.

Two-phase SPMD design over 8 NeuronCores, bf16 datapath:
  phase A: 3x3 texture-martingale statistics, sharded (batch x channel-half).
           THETA=1 collapses the martingale map to an affine rescale folded
           into the conv weights; the energy/entropy box filters commute
           with the 1x1 conv and are folded into the bilinear-resize
           matrices on host; the contrast feature is 8/9 + O(eps/sqrt(S2))
           and is folded into the per-channel constant. Homogeneity uses
           sum|x-mu| = 2*(sum_k max(x_k, mu) - S1), with the 9-tap sum done
           on the tensor engine via identity-matmul PSUM accumulation.
  phase B: fused 1x1 conv + layernorm (ones-matmul trick) + 8-head
           attention, all-bf16 matmuls, sharded (batch x query-token-half)
           with host-side token rolling. The softmax denominator falls out
           of the attnV matmul via a ones-column appended to V (row 65).
"""

import numpy as np

import concourse.bass as bass
import concourse.tile as tile
import concourse.mybir as mybir
from concourse import bacc

F32 = mybir.dt.float32
BF = mybir.dt.bfloat16
AF = mybir.ActivationFunctionType
ALU = mybir.AluOpType
EPS = 1e-6
E5 = float(np.exp(-0.5))

NCORES = 8
H = W = 64
HW = H * W
S = 1024         # tokens in phase B
D = 512
NH = 8
DH = 64

# 3x3 taps in torch-unfold order; (dh, dw)
TAPS = [(dh, dw) for dh in range(3) for dw in range(3)]


def new_bass():
    return bacc.Bacc("TRN2", target_bir_lowering=False, debug=False,
                     num_devices=NCORES)


def compile_with_pinned_tables(nc):
    """Restrict ACT table-set choice to {natural_log_exp_and_others}
    (positions preserved) so Ln/Exp never thrash sets."""
    import concourse.bacc as bacc_mod
    keep = {"natural_log_exp_and_others"}
    orig = bacc_mod.get_activation_tables

    def pinned(arch):
        return {name: (fns if name in keep else set())
                for name, fns in orig(arch).items()}

    bacc_mod.get_activation_tables = pinned
    try:
        nc.compile()
    finally:
        bacc_mod.get_activation_tables = orig
    return nc


# --------------------------------------------------------------------------
# Phase A kernel: per-core 128 channels, full 64x64 spatial.
# Outputs (per core, all [12, 4096] f32, pre-downsample):
#   contribA = sum_c Wh'*hom
#   qe       = sum_c We'*x^2       (box9 + resize applied on host)
#   qent     = sum_c Wen'*x*ln(x+eps)
# --------------------------------------------------------------------------
def build_phase_a():
    nc = new_bass()
    xp_in = nc.dram_tensor("xp", [128, 66, 68], BF, kind="ExternalInput")
    xp2_in = nc.dram_tensor("xp2", [128, 66, 68], BF, kind="ExternalInput")
    w_in = nc.dram_tensor("wstack", [128, 36], BF, kind="ExternalInput")
    id_in = nc.dram_tensor("ident", [128, 256], BF, kind="ExternalInput")
    o_a = nc.dram_tensor("contribA", [12, HW], F32, kind="ExternalOutput")
    o_e = nc.dram_tensor("qe", [12, HW], F32, kind="ExternalOutput")
    o_n = nc.dram_tensor("qent", [12, HW], F32, kind="ExternalOutput")

    with tile.TileContext(nc) as tc:
        with (
            tc.tile_pool(name="const", bufs=1) as constp,
            tc.tile_pool(name="big", bufs=1) as bigp,
            tc.tile_pool(name="mx", bufs=20) as mxp,
            tc.tile_pool(name="scr", bufs=3) as scrp,
            tc.tile_pool(name="ps", bufs=6, space="PSUM") as psp,
        ):
            wst = constp.tile([128, 36], BF, tag="w")
            nc.sync.dma_start(out=wst, in_=w_in[:, :])
            idt = constp.tile([128, 256], BF, tag="idt")
            nc.sync.dma_start(out=idt, in_=id_in[:, :])
            ident = idt[:, 0:128]
            nident9 = idt[:, 128:256]
            c_eps = constp.tile([128, 1], F32, tag="c_eps")
            nc.vector.memset(c_eps, EPS)

            xbf = bigp.tile([128, 66, 68], BF, tag="xbf")
            nc.sync.dma_start(out=xbf, in_=xp_in[:, :, :])
            xbf2 = bigp.tile([128, 66, 68], BF, tag="xbf2")
            nc.sync.dma_start(out=xbf2, in_=xp2_in[:, :, :])

            # scalar: x^2 (borders stay zero), ln(x+eps)
            x2 = bigp.tile([128, 66, 68], BF, tag="x2")
            nc.scalar.activation(out=x2, in_=xbf, func=AF.Square)
            lx = bigp.tile([128, HW], BF, tag="lx")
            nc.scalar.activation(out=lx, in_=xbf[:, 1:65, 1:65], func=AF.Ln,
                                 bias=c_eps)
            # gpsimd: xlx = x * ln(x+eps)
            xlx = bigp.tile([128, HW], BF, tag="xlx")
            nc.gpsimd.tensor_tensor(out=xlx, in0=xbf[:, 1:65, 1:65], in1=lx,
                                    op=ALU.mult)

            # DVE: separable box row-sums then col-sums (bf16, 2x aligned)
            tmpb = bigp.tile([128, 66, 64], BF, tag="tmpb")
            nc.vector.tensor_tensor(out=tmpb, in0=xbf[:, :, 0:64],
                                    in1=xbf[:, :, 2:66], op=ALU.add)
            bwb = bigp.tile([128, 66, 64], BF, tag="bwb")
            nc.vector.tensor_tensor(out=bwb, in0=tmpb, in1=xbf2[:, :, 2:66],
                                    op=ALU.add)
            s1a = bigp.tile([128, HW], BF, tag="s1a")
            nc.vector.tensor_tensor(out=s1a, in0=bwb[:, 0:64, :],
                                    in1=bwb[:, 2:66, :], op=ALU.add)
            s1 = bigp.tile([128, HW], BF, tag="s1")
            nc.vector.tensor_tensor(out=s1, in0=s1a, in1=bwb[:, 1:65, :],
                                    op=ALU.add)
            mu = bigp.tile([128, HW], BF, tag="mu")
            nc.vector.tensor_scalar_mul(mu, s1, 1.0 / 9.0)

            prev_contrib = None  # (ps3 tile, chunk j) pending contribA matmul
            for j in range(8):
                cn = slice(512 * j, 512 * (j + 1))
                # 9 maxes on DVE for this chunk
                ms = []
                for d, (dh, dw) in enumerate(TAPS):
                    src, c0 = (xbf2, 2) if dw == 1 else (xbf, dw)
                    m = mxp.tile([128, 512], BF, tag="m", name=f"m{j}_{d}")
                    nc.vector.tensor_tensor(
                        out=m, in0=src[:, 8 * j + dh:8 * j + 8 + dh, c0:c0 + 64],
                        in1=mu[:, cn], op=ALU.max)
                    ms.append(m)
                # PE: G = sum_d m_d - 9*mu  (identity-matmul accumulation)
                psg = psp.tile([128, 512], F32, tag="psg", bufs=2,
                               name=f"psg{j}")
                for d in range(9):
                    nc.tensor.matmul(psg, ident, ms[d], start=(d == 0),
                                     stop=False)
                nc.tensor.matmul(psg, nident9, mu[:, cn], start=False,
                                 stop=True)
                # PE: qe / qent contractions into shared psum tile
                ps3 = psp.tile([76, 512], F32, tag="ps3", bufs=3,
                               name=f"ps3{j}")
                nc.tensor.matmul(ps3[0:12, :], wst[:, 0:12],
                                 x2[:, 1 + 8 * j:9 + 8 * j, 1:65],
                                 start=True, stop=True)
                nc.tensor.matmul(ps3[32:44, :], wst[:, 12:24], xlx[:, cn],
                                 start=True, stop=True)
                # scalar: hom = exp(-ln(1 + (2/9) G))
                u = scrp.tile([128, 512], F32, tag="u", name=f"u{j}")
                nc.scalar.activation(out=u, in_=psg, func=AF.Ln,
                                     scale=2.0 / 9.0, bias=1.0)
                homc = scrp.tile([128, 512], BF, tag="hom", name=f"hom{j}")
                nc.scalar.activation(out=homc, in_=u, func=AF.Exp, scale=-1.0)
                # defer contribA matmul one chunk so PE never stalls on scalar
                if prev_contrib is not None:
                    pps3, pj, phom = prev_contrib
                    nc.tensor.matmul(pps3[64:76, :], wst[:, 24:36], phom,
                                     start=True, stop=True)
                    _emit_a_out(nc, scrp, pps3, pj, o_e, o_n, o_a)
                prev_contrib = (ps3, j, homc)
            pps3, pj, phom = prev_contrib
            nc.tensor.matmul(pps3[64:76, :], wst[:, 24:36], phom,
                             start=True, stop=True)
            _emit_a_out(nc, scrp, pps3, pj, o_e, o_n, o_a)

    compile_with_pinned_tables(nc)
    return nc


def _emit_a_out(nc, scrp, ps3, j, o_e, o_n, o_a):
    cn = slice(512 * j, 512 * (j + 1))
    st = scrp.tile([76, 512], mybir.dt.float32, tag="stg", bufs=3,
                   name=f"stg{j}")
    nc.scalar.activation(out=st, in_=ps3, func=AF.Copy)
    nc.gpsimd.dma_start(out=o_e[:, cn], in_=st[0:12, :])
    nc.gpsimd.dma_start(out=o_n[:, cn], in_=st[32:44, :])
    nc.gpsimd.dma_start(out=o_a[:, cn], in_=st[64:76, :])


# --------------------------------------------------------------------------
# Host-side helpers
# --------------------------------------------------------------------------
def resize_mat(out_size, in_size):
    scale = out_size / in_size
    sample_f = (np.arange(out_size) + 0.5) / scale - 0.5
    x = np.abs(sample_f[:, None] - np.arange(in_size)[None, :]) * scale
    w = np.maximum(0.0, 1.0 - x)
    w = w / w.sum(axis=1, keepdims=True)
    return w  # float64 [out, in]


def prep_a(x, reduce_w):
    """Returns in_maps (list of 8 dicts) for phase A."""
    import ml_dtypes
    BFnp = ml_dtypes.bfloat16
    E = E5
    ident = np.zeros((128, 256), BFnp)
    ident[:, 0:128] = np.eye(128, dtype=np.float32)
    ident[:, 128:256] = (-9.0) * np.eye(128, dtype=np.float32)
    in_maps = []
    for core in range(NCORES):
        b, half = core // 2, core % 2
        c0 = half * 128
        cols = 4 * (c0 + np.arange(128))
        wstack = np.stack([
            reduce_w[:, cols + 1].T * (E / 9.0),      # energy (pre-box)
            reduce_w[:, cols + 2].T * (-E / 9.0),     # entropy (pre-box)
            reduce_w[:, cols + 3].T * E,              # homogeneity
        ], axis=1).reshape(128, 36).astype(np.float32)
        xb = np.asarray(x[b, c0:c0 + 128], np.float32).astype(BFnp)
        xp = np.zeros((128, 66, 68), BFnp)
        xp[:, 1:65, 1:65] = xb
        xp2 = np.zeros((128, 66, 68), BFnp)
        xp2[:, 1:65, 2:66] = xb
        in_maps.append({
            "xp": xp,
            "xp2": xp2,
            "wstack": np.ascontiguousarray(wstack.astype(BFnp)),
            "ident": ident,
        })
    return in_maps


def combine_a(results, reduce_w, reduce_b):
    """results: list of 8 dicts with contribA/qe/qent [12, 4096].
    Returns m_up [4, 12, 32, 32] float32."""
    R = resize_mat(32, 64)
    Bm = (np.abs(np.arange(64)[:, None] - np.arange(64)[None, :]) <= 1).astype(np.float64)
    RB = R @ Bm
    rw = reduce_w.astype(np.float64)
    const_o = (reduce_b.astype(np.float64) + EPS * E5 * rw.sum(axis=1)
               + (8.0 / 9.0) * E5 * rw[:, 0::4].sum(axis=1))
    m_up = np.zeros((4, 12, 32, 32), np.float64)
    for b in range(4):
        A = (results[2 * b]["contribA"].astype(np.float64)
             + results[2 * b + 1]["contribA"].astype(np.float64)).reshape(12, 64, 64)
        qe = (results[2 * b]["qe"].astype(np.float64)
              + results[2 * b + 1]["qe"].astype(np.float64)).reshape(12, 64, 64)
        qn = (results[2 * b]["qent"].astype(np.float64)
              + results[2 * b + 1]["qent"].astype(np.float64)).reshape(12, 64, 64)
        m_up[b] = (np.einsum("ij,ojk,lk->oil", R, A, R)
                   + np.einsum("ij,ojk,lk->oil", RB, qe, RB)
                   + np.einsum("ij,ojk,lk->oil", RB, qn, RB)
                   + const_o[:, None, None])
    return m_up.astype(np.float32)


# --------------------------------------------------------------------------
# Phase B kernel: fuse conv + LN + 8-head attention for one batch element,
# query-token half = tokens [0:512] (host pre-rolls tokens per core).
# --------------------------------------------------------------------------
def build_phase_b():
    nc = new_bass()
    xn_in = nc.dram_tensor("xn", [128, 4, S], BF, kind="ExternalInput")
    mu_in = nc.dram_tensor("mu12", [12, S], BF, kind="ExternalInput")
    wxt_in = nc.dram_tensor("wxt", [128, 4, 512], BF, kind="ExternalInput")
    wmt_in = nc.dram_tensor("wmt", [12, 512], BF, kind="ExternalInput")
    wpt_in = nc.dram_tensor("wpt", [12, 512], BF, kind="ExternalInput")
    wqt_in = nc.dram_tensor("wqt", [128, 4, 512], BF, kind="ExternalInput")
    wkt_in = nc.dram_tensor("wkt", [128, 4, 512], BF, kind="ExternalInput")
    wvt_in = nc.dram_tensor("wvt", [128, 4, 512], BF, kind="ExternalInput")
    wot_in = nc.dram_tensor("wot", [128, 4, 512], BF, kind="ExternalInput")
    bp_in = nc.dram_tensor("biasp", [128, 20], F32, kind="ExternalInput")
    bv_in = nc.dram_tensor("bvrow", [1, 512], BF, kind="ExternalInput")
    ones_in = nc.dram_tensor("ones128", [128, 128], BF, kind="ExternalInput")
    fout = nc.dram_tensor("fout", [512, 512], F32, kind="ExternalOutput")

    SQ = 512  # query tokens per core
    with tile.TileContext(nc) as tc:
        with (
            tc.tile_pool(name="wp", bufs=1) as wp,
            tc.tile_pool(name="dat", bufs=1) as dat,
            tc.tile_pool(name="scr", bufs=2) as scr,
            tc.tile_pool(name="ps", bufs=8, space="PSUM") as psp,
        ):
            def load(pool, dram, shape, tag, dtype=BF):
                t = pool.tile(shape, dtype, tag=tag)
                nc.sync.dma_start(out=t, in_=dram[tuple(slice(None) for _ in shape)])
                return t

            # fuse-conv inputs first so the PE can start ASAP
            xn_t = load(dat, xn_in, [128, 4, S], "xn")
            wxt_t = load(wp, wxt_in, [128, 4, 512], "wxt")
            wmt_t = load(wp, wmt_in, [12, 512], "wmt")
            mu_t = load(wp, mu_in, [12, S], "mu")
            biasp = load(wp, bp_in, [128, 20], "biasp", dtype=F32)
            wpt_t = load(wp, wpt_in, [12, 512], "wpt")
            ones128 = load(wp, ones_in, [128, 128], "ones128")
            wkt_t = load(wp, wkt_in, [128, 4, 512], "wkt")
            wqt_t = load(wp, wqt_in, [128, 4, 512], "wqt")
            wvt_t = load(wp, wvt_in, [128, 4, 512], "wvt")
            bvrow = load(wp, bv_in, [1, 512], "bvrow")
            wot_t = load(wp, wot_in, [128, 4, 512], "wot")
            ones1 = ones128[0:1, :]
            c_ln5 = wp.tile([128, 1], F32, tag="c_ln5")
            nc.vector.memset(c_ln5, 1e-5)

            def wblk(t, kb, mb):
                return t[:, kb, 128 * mb:128 * (mb + 1)]

            # ---- fuse conv: v_sb = xf (V source), qkp = xf + proj@mu + proj_b
            v_sb = [[None, None] for _ in range(4)]
            qkp = [[None, None] for _ in range(4)]
            for mb in range(4):
                psA = [psp.tile([128, 512], F32, tag="ps", bufs=8,
                                name=f"psA{mb}_{i}") for i in range(2)]
                for kb in range(4):
                    for nb in range(2):
                        nc.tensor.matmul(
                            psA[nb], wblk(wxt_t, kb, mb),
                            xn_t[:, kb, 512 * nb:512 * (nb + 1)],
                            start=(kb == 0), stop=False)
                for nb in range(2):
                    nc.tensor.matmul(
                        psA[nb], wmt_t[:, 128 * mb:128 * (mb + 1)],
                        mu_t[:, 512 * nb:512 * (nb + 1)],
                        start=False, stop=True)
                for nb in range(2):
                    psB = psp.tile([128, 512], F32, tag="ps", bufs=8)
                    nc.tensor.matmul(
                        psB, wpt_t[:, 128 * mb:128 * (mb + 1)],
                        mu_t[:, 512 * nb:512 * (nb + 1)], start=True, stop=True)
                    v = dat.tile([128, 512], BF, tag="v_sb", bufs=8)
                    nc.scalar.activation(out=v, in_=psA[nb], func=AF.Identity,
                                         bias=biasp[:, mb:mb + 1])
                    v_sb[mb][nb] = v
                    q = dat.tile([128, 512], BF, tag="qkp", bufs=8)
                    nc.vector.scalar_tensor_tensor(
                        out=q, in0=psB, scalar=biasp[:, 4 + mb:5 + mb], in1=v,
                        op0=ALU.add, op1=ALU.add)
                    qkp[mb][nb] = q

            # ---- LN over channels (partition dim) via ones-matmul
            psmu = [psp.tile([128, 512], F32, tag="ps", bufs=8,
                             name=f"psmu{i}") for i in range(2)]
            pss2 = [psp.tile([128, 512], F32, tag="ps", bufs=8,
                             name=f"pss2{i}") for i in range(2)]
            for nb in range(2):
                for mb in range(4):
                    nc.tensor.matmul(psmu[nb], ones128, qkp[mb][nb],
                                     start=(mb == 0), stop=(mb == 3))
                for mb in range(4):
                    sq = scr.tile([128, 512], BF, tag="sq", bufs=2)
                    nc.vector.tensor_tensor(out=sq, in0=qkp[mb][nb],
                                            in1=qkp[mb][nb], op=ALU.mult)
                    nc.tensor.matmul(pss2[nb], ones128, sq,
                                     start=(mb == 0), stop=(mb == 3))
            mu_sb, rstd = [], []
            for nb in range(2):
                m = dat.tile([128, 512], F32, tag="mu_sb", bufs=2)
                nc.vector.tensor_scalar_mul(m, psmu[nb], 1.0 / 512.0)
                mu_sb.append(m)
                musq = scr.tile([128, 512], F32, tag="musq", bufs=2)
                nc.vector.tensor_tensor(out=musq, in0=m, in1=m, op=ALU.mult)
                vt = scr.tile([128, 512], F32, tag="vt", bufs=2)
                nc.vector.scalar_tensor_tensor(
                    out=vt, in0=musq, scalar=-512.0, in1=pss2[nb],
                    op0=ALU.mult, op1=ALU.add)
                lnv = scr.tile([128, 512], F32, tag="lnv", bufs=2)
                nc.scalar.activation(out=lnv, in_=vt, func=AF.Ln,
                                     scale=1.0 / 512.0, bias=c_ln5)
                r = dat.tile([128, 512], BF, tag="rstd", bufs=2)
                nc.scalar.activation(out=r, in_=lnv, func=AF.Exp, scale=-0.5)
                rstd.append(r)
            qkl = [[None, None] for _ in range(4)]
            for nb in range(2):
                for mb in range(4):
                    cen = dat.tile([128, 512], BF, tag="qkl", bufs=8)
                    nc.vector.scalar_tensor_tensor(
                        out=cen, in0=mu_sb[nb], scalar=-1.0,
                        in1=qkp[mb][nb], op0=ALU.mult, op1=ALU.add)
                    nc.gpsimd.tensor_tensor(out=cen, in0=cen, in1=rstd[nb],
                                            op=ALU.mult)
                    qkl[mb][nb] = cen

            # ---- V in [token, head, dh+1] layout; col 64 of each head = 1.0
            # (the ones column makes the softmax denominator fall out of the
            # attnV matmul as psum row 64)
            def vproj(tb):
                psv = psp.tile([128, 512], F32, tag="ps", bufs=8)
                nc.tensor.matmul(psv, ones1, bvrow, start=True, stop=False)
                for kb in range(4):
                    nc.tensor.matmul(
                        psv,
                        v_sb[kb][tb // 4][:, 128 * (tb % 4):128 * (tb % 4 + 1)],
                        wvt_t[:, kb, :], start=False, stop=(kb == 3))
                vt2 = dat.tile([128, 8, 65], BF, tag="v2_sb", bufs=8)
                nc.vector.tensor_copy(vt2[:, :, 0:64], psv)
                nc.vector.memset(vt2[:, :, 64:65], 1.0)
                return vt2

            v2_sb = [None] * 8
            for tb in range(4):
                v2_sb[tb] = vproj(tb)

            # ---- K (all tokens), Q (tokens 0:512), scores+exp interleaved
            k_sb, q_sb = [None] * 4, [None] * 4

            def kqproj(hp):
                kt = dat.tile([128, S], BF, tag="k_sb", bufs=4)
                for nb in range(2):
                    psk = psp.tile([128, 512], F32, tag="ps", bufs=8)
                    for kb in range(4):
                        nc.tensor.matmul(psk, wblk(wkt_t, kb, hp),
                                         qkl[kb][nb], start=(kb == 0),
                                         stop=(kb == 3))
                    nc.vector.tensor_scalar(
                        out=kt[:, 512 * nb:512 * (nb + 1)], in0=psk,
                        scalar1=biasp[:, 12 + hp:13 + hp], scalar2=None,
                        op0=ALU.add)
                k_sb[hp] = kt
                psq = psp.tile([128, 512], F32, tag="ps", bufs=8)
                for kb in range(4):
                    nc.tensor.matmul(psq, wblk(wqt_t, kb, hp),
                                     qkl[kb][0], start=(kb == 0), stop=(kb == 3))
                qt = dat.tile([128, 512], BF, tag="q_sb", bufs=4)
                nc.scalar.activation(out=qt, in_=psq, func=AF.Identity,
                                     bias=biasp[:, 8 + hp:9 + hp])
                q_sb[hp] = qt

            e_sb = [[None] * 8 for _ in range(NH)]

            def scores(h):
                hp, off = h // 2, 64 * (h % 2)
                for tb in range(8):
                    pss = psp.tile([128, SQ], F32, tag="ps", bufs=8)
                    nc.tensor.matmul(
                        pss, k_sb[hp][off:off + 64, 128 * tb:128 * (tb + 1)],
                        q_sb[hp][off:off + 64, :], start=True, stop=True)
                    e = dat.tile([128, SQ], BF, tag="e_sb", bufs=64)
                    nc.scalar.activation(out=e, in_=pss, func=AF.Exp)
                    e_sb[h][tb] = e

            kqproj(0)
            scores(0)
            scores(1)
            for tb in range(4, 8):
                v2_sb[tb] = vproj(tb)
            for hp in range(1, 4):
                kqproj(hp)
                scores(2 * hp)
                scores(2 * hp + 1)

            # ---- attention output per head; denominator = psum row 64
            o_sb = [dat.tile([128, SQ], BF, tag="o_sb", bufs=4,
                             name=f"o_sb{i}") for i in range(4)]
            with nc.allow_low_precision("bf16 softmax denominators"):
                for h in range(NH):
                    hp, off = h // 2, 64 * (h % 2)
                    pso = psp.tile([65, SQ], F32, tag="ps", bufs=8)
                    for tb in range(8):
                        nc.tensor.matmul(pso, v2_sb[tb][:, h, :],
                                         e_sb[h][tb], start=(tb == 0),
                                         stop=(tb == 7))
                    rinv = scr.tile([1, SQ], BF, tag="rinv", bufs=2)
                    nc.vector.reciprocal(rinv, pso[64:65, :])
                    rb = scr.tile([64, SQ], BF, tag="rb", bufs=2)
                    nc.gpsimd.partition_broadcast(rb, rinv)
                    nc.vector.tensor_tensor(out=o_sb[hp][off:off + 64, :],
                                            in0=pso[0:64, :], in1=rb,
                                            op=ALU.mult)

            # ---- output projection
            for mb in range(4):
                psf = psp.tile([128, SQ], F32, tag="ps", bufs=8)
                for hp in range(4):
                    nc.tensor.matmul(psf, wblk(wot_t, hp, mb),
                                     o_sb[hp], start=(hp == 0), stop=(hp == 3))
                fo = scr.tile([128, SQ], F32, tag="fo", bufs=2)
                nc.scalar.activation(out=fo, in_=psf, func=AF.Identity,
                                     bias=biasp[:, 16 + mb:17 + mb])
                nc.sync.dma_start(out=fout[128 * mb:128 * (mb + 1), :], in_=fo)

    compile_with_pinned_tables(nc)
    return nc


def prep_b(x_next, m_up, fuse_w, fuse_b, proj_w, proj_b, ln_g, ln_b,
           in_proj_w, in_proj_b, out_proj_w, out_proj_b):
    import ml_dtypes
    BFnp = ml_dtypes.bfloat16
    wq, wk, wv = in_proj_w[:512], in_proj_w[512:1024], in_proj_w[1024:]
    bq, bk, bv = in_proj_b[:512], in_proj_b[512:1024], in_proj_b[1024:]
    f32 = np.float32
    wq2 = (wq * ln_g[None, :]) / 8.0
    bq2 = (bq + wq @ ln_b) / 8.0
    wk2 = wk * ln_g[None, :]
    bk2 = bk + wk @ ln_b

    def pack(v):
        return np.ascontiguousarray(v.reshape(4, 128).T).astype(f32)

    def blocks(w):
        # [512 in, 512 out] -> [128, 4, 512]: partition p, contraction block
        # kb, out col — matches the SBUF tile layout for a single DMA
        return np.ascontiguousarray(
            w.T.reshape(4, 128, 512).transpose(1, 0, 2)).astype(BFnp)

    biasp = np.concatenate([pack(fuse_b), pack(proj_b), pack(bq2), pack(bk2),
                            pack(out_proj_b)], axis=1)
    shared = {
        "wxt": blocks(fuse_w[:, :512]),
        "wmt": np.ascontiguousarray(fuse_w[:, 512:].T).astype(BFnp),
        "wpt": np.ascontiguousarray(proj_w.T).astype(BFnp),
        "wqt": blocks(wq2),
        "wkt": blocks(wk2),
        "wvt": blocks(wv),
        "wot": blocks(out_proj_w),
        "biasp": biasp,
        "bvrow": np.ascontiguousarray(bv[None, :]).astype(BFnp),
        "ones128": np.ones((128, 128), BFnp),
    }
    in_maps = []
    for core in range(NCORES):
        b, half = core // 2, core % 2
        xn = x_next[b].reshape(512, S)
        mu = m_up[b].reshape(12, S)
        if half == 1:
            xn = np.roll(xn, -512, axis=1)
            mu = np.roll(mu, -512, axis=1)
        m = dict(shared)
        m["xn"] = np.ascontiguousarray(
            xn.reshape(4, 128, S).transpose(1, 0, 2)).astype(BFnp)
        m["mu12"] = np.ascontiguousarray(mu).astype(BFnp)
        in_maps.append(m)
    return in_maps


def combine_b(results):
    out = np.zeros((4, 512, S), np.float32)
    for core in range(NCORES):
        b, half = core // 2, core % 2
        out[b][:, 512 * half:512 * (half + 1)] = results[core]["fout"]
    return out.reshape(4, 512, 32, 32)


# --------------------------------------------------------------------------
# Entry point
# --------------------------------------------------------------------------
_CACHE = {}


def _get_programs():
    if "a" not in _CACHE:
        _CACHE["a"] = build_phase_a()
        _CACHE["b"] = build_phase_b()
    return _CACHE["a"], _CACHE["b"]


def kernel(x, x_next, reduce_w, reduce_b, fuse_w, fuse_b, proj_w, proj_b,
           ln_g, ln_b, in_proj_w, in_proj_b, out_proj_w, out_proj_b):
    from concourse.bass_utils import run_bass_kernel_spmd

    x = np.asarray(x, np.float32)
    x_next = np.asarray(x_next, np.float32)
    nc_a, nc_b = _get_programs()

    in_a = prep_a(x, np.asarray(reduce_w, np.float32))
    res_a = run_bass_kernel_spmd(nc_a, in_a, core_ids=list(range(NCORES)))
    m_up = combine_a(res_a.results, np.asarray(reduce_w, np.float64),
                     np.asarray(reduce_b, np.float64))

    in_b = prep_b(x_next, m_up, *(np.asarray(t, np.float32) for t in (
        fuse_w, fuse_b, proj_w, proj_b, ln_g, ln_b,
        in_proj_w, in_proj_b, out_proj_w, out_proj_b)))
    res_b = run_bass_kernel_spmd(nc_b, in_b, core_ids=list(range(NCORES)))
    return combine_b(res_b.results)
